# revision 3
# baseline (speedup 1.0000x reference)
"""DeepGCN (3-layer GCNConv + BN + ReLU) on 8 Trainium2 NeuronCores.

Strategy (graph/data parallel, dst-sharded):
 - Nodes padded to NPAD=50176 = 8 cores x 6272 rows = 392 blocks of 128.
 - Edges (incl. self-loops) partitioned by destination core/block.
 - Per layer: sharded feature matmul -> AllGather of the (pre-scaled,
   bf16) message operand -> per-dst-block message passing:
     gather source rows (GPSIMD dma_gather, 256B rows) ->
     segment-sum via PE matmul with a 0/1 selection matrix built on DVE
     (is_equal of dst_local against an iota row) accumulated in PSUM ->
     ACT epilogue (Relu/scale by D^-1/2, BN affine folded into weights).
 - Normalization dis[src]*dis[dst] is separable: pre-scale the operand
   rows by dis, post-scale the segment sum by dis.
 - BN: y = s*conv + t with s,t folded into W (columns) and a rank-1 bias
   matmul (invdis x Q) so that dis*(segsum + invdis*Q) = dis*segsum + Q.
 - int16 gather indices (max 32767) => per-block edges split into an
   A call (src < 32768, base row 0) and a B call (src >= 17408, base row
   17408); the middle band is assigned to whichever side has slots.
"""

import numpy as np
import ml_dtypes

import concourse.bacc as bacc
import concourse.mybir as mybir
import concourse.tile as tile
from concourse.bass_utils import run_bass_kernel_spmd

BF16 = ml_dtypes.bfloat16

N = 50000
DIN = 512
H1 = 128
H2 = 64
NCLS = 2
EPS = 1e-5

NCORES = 8
P = 128
NB = 49                  # dst blocks per core
SHN = NB * P             # 6272 nodes per core
NPAD = NCORES * SHN      # 50176
GRPB = 7                 # blocks per gather group
NGRP = NB // GRPB        # 7
BBASE = 17408            # base row of the B gather window
AHI = 32768              # A window is [0, 32768)
NQ = 4                   # SWDGE queues for dma_gather

_cache: dict = {}
_dbg_names: dict = {}


# --------------------------------------------------------------------------
# host-side preprocessing
# --------------------------------------------------------------------------

def _make_schedule(edge_index):
    """Partition edges by (core, block), pick shared per-block chunk counts.

    Returns sched dict:
      kA, kB: [NB] shared chunk counts per block (A / B gather calls)
      per-core padded edge streams: gidx (int16 gather idx), dloc (f32 dst
      local, -1 for padding), laid out group-major:
      [g0: A-chunks(b0..b6) | B-chunks(b0..b6)] [g1: ...]
    """
    src = np.concatenate([edge_index[0], np.arange(N, dtype=np.int64)])
    dst = np.concatenate([edge_index[1], np.arange(N, dtype=np.int64)])
    src = src.astype(np.int64)
    dst = dst.astype(np.int64)

    core = dst // SHN
    blk = (dst % SHN) // P
    dl = (dst % P).astype(np.int32)
    key = (core * NB + blk).astype(np.int64)
    order = np.argsort(key, kind="stable")
    s_src = src[order]
    s_dl = dl[order]
    s_key = key[order]
    bounds = np.searchsorted(s_key, np.arange(NCORES * NB + 1))

    # segment (c,b): s_src[bounds[c*NB+b]:bounds[c*NB+b+1]]
    # per-core low/mid/high counts per block
    nlow = np.zeros((NCORES, NB), np.int64)
    nhigh = np.zeros((NCORES, NB), np.int64)
    ntot = np.zeros((NCORES, NB), np.int64)
    segs = {}
    for c in range(NCORES):
        for b in range(NB):
            i0, i1 = bounds[c * NB + b], bounds[c * NB + b + 1]
            ss = s_src[i0:i1]
            dd = s_dl[i0:i1]
            segs[(c, b)] = (ss, dd)
            ntot[c, b] = i1 - i0
            nlow[c, b] = int((ss < BBASE).sum())
            nhigh[c, b] = int((ss >= AHI).sum())

    K = np.maximum(1, -(-ntot.max(axis=0) // P))          # ceil
    kA_min = -(-nlow.max(axis=0) // P)
    kA_max = K - (-(-nhigh.max(axis=0) // P))
    assert (kA_min <= kA_max).all(), "A/B split infeasible"
    kA = np.clip((K * 2) // 3, kA_min, kA_max)
    kB = K - kA

    # build per-core streams
    gidx_cores, dloc_cores = [], []
    for c in range(NCORES):
        gparts, dparts = [], []
        for g in range(NGRP):
            blocks = range(g * GRPB, (g + 1) * GRPB)
            for side in (0, 1):
                for b in blocks:
                    ss, dd = segs[(c, b)]
                    low = ss < BBASE
                    high = ss >= AHI
                    mid = ~low & ~high
                    slots_a = int(kA[b]) * P
                    # A: all low + as many mid as fit
                    mid_idx = np.nonzero(mid)[0]
                    a_take = min(len(mid_idx), slots_a - int(low.sum()))
                    assert a_take >= 0
                    a_sel = np.concatenate([np.nonzero(low)[0], mid_idx[:a_take]])
                    b_sel = np.concatenate([mid_idx[a_take:], np.nonzero(high)[0]])
                    assert len(b_sel) <= int(kB[b]) * P
                    if side == 0:
                        sel, slots, base = a_sel, slots_a, 0
                    else:
                        sel, slots, base = b_sel, int(kB[b]) * P, BBASE
                    idx = ss[sel] - base
                    dloc = dd[sel].astype(np.float32)
                    padn = slots - len(sel)
                    idx = np.concatenate([idx, np.zeros(padn, np.int64)])
                    dloc = np.concatenate([dloc, -np.ones(padn, np.float32)])
                    assert (idx >= 0).all() and (idx < 32768).all()
                    gparts.append(idx.astype(np.int16))
                    dparts.append(dloc)
        gidx_cores.append(np.concatenate(gparts))
        dloc_cores.append(np.concatenate(dparts))

    T = int(K.sum())  # total chunks per core
    return {
        "kA": kA.astype(int).tolist(),
        "kB": kB.astype(int).tolist(),
        "T": T,
        "gidx": gidx_cores,
        "dloc": dloc_cores,
    }


def _prep_inputs(sched, x, w1, b1, g1, beta1, m1, v1,
                 w2, b2, g2, beta2, m2, v2, w3, b3):
    s1 = g1 / np.sqrt(v1 + EPS)
    t1 = beta1 - m1 * s1
    s2 = g2 / np.sqrt(v2 + EPS)
    t2 = beta2 - m2 * s2
    q1 = (s1 * b1 + t1).astype(np.float32)[None, :]
    q2 = (s2 * b2 + t2).astype(np.float32)[None, :]
    w1s = (w1 * s1[None, :]).astype(np.float32)
    w2s = (w2 * s2[None, :]).astype(np.float32)

    # degrees on A+I (in-degree by dst), dis = deg^-1/2
    deg = np.zeros(NPAD, np.float32)
    cnt = np.bincount(sched["dst_all"], minlength=N).astype(np.float32)
    deg[:N] = cnt
    dis = np.where(deg > 0, 1.0 / np.sqrt(np.maximum(deg, 1e-30)), 0.0)
    invdis = np.sqrt(deg)

    xp = np.zeros((NPAD, DIN), np.float32)
    xp[:N] = x

    T = sched["T"]
    iota = np.broadcast_to(np.arange(P, dtype=np.float32), (P, P))

    common = {
        "w1s": np.ascontiguousarray(
            w1s.reshape(4, P, H1).transpose(1, 0, 2)).astype(BF16),
        "w2s": w2s.astype(BF16),
        "w3": w3.astype(np.float32),
        "q1": q1,
        "q2": q2,
        "b3r": b3.astype(np.float32)[None, :],
        "ones": np.ones((1, P), np.float32),
        "iota": iota.astype(BF16),
    }

    in_maps = []
    for c in range(NCORES):
        off = c * SHN
        xc = xp[off:off + SHN].reshape(NB, P, 4, P)      # [b, n, t, p]
        xtt = np.ascontiguousarray(xc.transpose(0, 3, 2, 1)).astype(BF16)
        disc = np.ascontiguousarray(dis[off:off + SHN].reshape(NB, P).T)
        ivd = np.ascontiguousarray(invdis[off:off + SHN])[None, :]
        gidx = sched["gidx"][c]
        dloc = sched["dloc"][c]
        idx_sb = np.tile(gidx.reshape(T * 8, 16).T, (8, 1))
        dstl = np.ascontiguousarray(dloc.reshape(T, P).T).astype(BF16)
        m = dict(common)
        m.update({
            "xtt": xtt,
            "disc": disc,
            "ivd": ivd,
            "idx": np.ascontiguousarray(idx_sb),
            "dstl": dstl,
        })
        in_maps.append(m)
    return in_maps


# --------------------------------------------------------------------------
# bass program
# --------------------------------------------------------------------------

def _build(sched, stages=99, no_coll=False, reps=None, mpv="full", nq=None,
           small_coll=False):
    reps = reps or {}
    nq = NQ if nq is None else nq
    NQL = nq
    dt = mybir.dt
    kA, kB, T = sched["kA"], sched["kB"], sched["T"]
    K = [a + b for a, b in zip(kA, kB)]
    KGmax = max(sum(K[g * GRPB:(g + 1) * GRPB]) for g in range(NGRP))
    KBmax = max(K)

    nc = bacc.Bacc("TRN2", target_bir_lowering=False, debug=False,
                   num_devices=NCORES, num_swdge_queues=nq)

    xtt = nc.dram_tensor("xtt", [NB, P, 4, P], dt.bfloat16, kind="ExternalInput")
    w1s = nc.dram_tensor("w1s", [P, 4, H1], dt.bfloat16, kind="ExternalInput")
    w2s = nc.dram_tensor("w2s", [P, H2], dt.bfloat16, kind="ExternalInput")
    w3 = nc.dram_tensor("w3", [H2, NCLS], dt.float32, kind="ExternalInput")
    q1 = nc.dram_tensor("q1", [1, H1], dt.float32, kind="ExternalInput")
    q2 = nc.dram_tensor("q2", [1, H2], dt.float32, kind="ExternalInput")
    b3r = nc.dram_tensor("b3r", [1, NCLS], dt.float32, kind="ExternalInput")
    ones = nc.dram_tensor("ones", [1, P], dt.float32, kind="ExternalInput")
    iota = nc.dram_tensor("iota", [P, P], dt.bfloat16, kind="ExternalInput")
    disc = nc.dram_tensor("disc", [P, NB], dt.float32, kind="ExternalInput")
    ivd = nc.dram_tensor("ivd", [1, SHN], dt.float32, kind="ExternalInput")
    dstl = nc.dram_tensor("dstl", [P, T], dt.bfloat16, kind="ExternalInput")
    idx = nc.dram_tensor("idx", [P, T * 8], dt.int16, kind="ExternalInput")
    outt = nc.dram_tensor("out", [SHN, NCLS], dt.float32, kind="ExternalOutput")

    Relu = mybir.ActivationFunctionType.Relu
    Copy = mybir.ActivationFunctionType.Copy
    rg = [list(range(NCORES))]

    with tile.TileContext(nc) as tc:
        with (
            tc.tile_pool(name="cst", bufs=1) as cst,
            tc.tile_pool(name="res", bufs=1) as res,
            tc.tile_pool(name="dram", bufs=1, space="DRAM") as dram,
            tc.tile_pool(name="work", bufs=3) as work,
            tc.tile_pool(name="gt", bufs=2) as gpool,
            tc.tile_pool(name="sp", bufs=3) as spool,
            tc.tile_pool(name="ps", bufs=2, space="PSUM") as pp,
        ):
            # ---- constants into SBUF ----
            def cload(ap_dram, shape, dtype, tag):
                t = cst.tile(shape, dtype, tag=tag)
                nc.sync.dma_start(out=t[:], in_=ap_dram)
                return t

            w1_t = cload(w1s[:], [P, 4, H1], dt.bfloat16, "w1")
            w2_t = cload(w2s[:], [P, H2], dt.bfloat16, "w2")
            w3_t = cload(w3[:], [H2, NCLS], dt.float32, "w3")
            q1_t = cload(q1[:], [1, H1], dt.float32, "q1")
            q2_t = cload(q2[:], [1, H2], dt.float32, "q2")
            b3_t = cload(b3r[:], [1, NCLS], dt.float32, "b3")
            on_t = cload(ones[:], [1, P], dt.float32, "on")
            io_t = cload(iota[:], [P, P], dt.bfloat16, "io")
            di_t = cload(disc[:], [P, NB], dt.float32, "di")
            iv_t = cload(ivd[:], [1, SHN], dt.float32, "iv")
            dl_t = cload(dstl[:], [P, T], dt.bfloat16, "dl")
            ix_t = cload(idx[:], [P, T * 8], dt.int16, "ix")
            from concourse.masks import make_identity
            idn_f = cst.tile([P, P], dt.float32, tag="idf")
            make_identity(nc, idn_f[:])
            idn_b = cst.tile([P, P], dt.bfloat16, tag="idb")
            make_identity(nc, idn_b[:])

            io3 = io_t[:].rearrange("p (a q) -> p a q", a=1)

            # persistent tiles
            A1 = res.tile([P, NB * P], dt.bfloat16, tag="a1")
            outacc = res.tile([P, NB * NCLS], dt.float32, tag="oa")

            # dram bounce buffers (ping-pong)
            shard = [dram.tile([SHN, P], dt.bfloat16, tag=f"shard{i}", name=f"shard{i}")
                     for i in range(2)]

            def allgather(si, fi):
                if no_coll:
                    # timeline-sim stand-in: local copy only (timing analysis)
                    nc.sync.dma_start(out=full[fi][0:SHN, :], in_=shard[si][:])
                    return
                if small_coll:
                    # timing probe: 16-row collective (results garbage)
                    nc.gpsimd.collective_compute(
                        "AllGather", mybir.AluOpType.bypass, replica_groups=rg,
                        ins=[shard[si][0:16, :]], outs=[full[fi][0:128, :]],
                    )
                    return
                nc.gpsimd.collective_compute(
                    "AllGather", mybir.AluOpType.bypass, replica_groups=rg,
                    ins=[shard[si][:]], outs=[full[fi][:]],
                )

            for _rr in range(reps.get("all", 1)):
              full = [
                  dram.tile([NPAD, P], dt.bfloat16, addr_space="Shared",
                            tag=f"full{_rr}_{i}", name=f"full{_rr}_{i}")
                  for i in range(3)
              ]
              nc.vector.memset(outacc[:], 0.0)
              # ---- phase M1: h1 = dis * (x @ W1s) -> shard0 ----
              # batched per group of GRPB blocks to amortize SP DMA issue cost
              with nc.named_scope("m1"):
                for _r in range(reps.get("m1", 1)):
                  for g in range(NGRP):
                      xt = work.tile([P, GRPB, 4, P], dt.bfloat16, tag="xtt")
                      nc.sync.dma_start(
                          out=xt[:], in_=xtt[g * GRPB:(g + 1) * GRPB]
                          .rearrange("b p t n -> p b t n"))
                      hb = work.tile([P, GRPB, H1], dt.bfloat16, tag="hb")
                      for j in range(GRPB):
                          b = g * GRPB + j
                          ps = pp.tile([P, H1], dt.float32, tag="mp")
                          for t in range(4):
                              nc.tensor.matmul(ps[:], xt[:, j, t, :],
                                               w1_t[:, t, :],
                                               start=(t == 0), stop=(t == 3))
                          nc.scalar.activation(hb[:, j, :], ps[:], Copy,
                                               scale=di_t[:, b:b + 1])
                      nc.sync.dma_start(
                          out=shard[0][g * GRPB * P:(g + 1) * GRPB * P, :]
                          .rearrange("(b p) h -> p b h", p=P),
                          in_=hb[:])
                  if stages >= 2:
                      allgather(0, 0)
                      for _r in range(reps.get("ag1", 1) - 1):
                          fx = dram.tile([NPAD, P], dt.bfloat16,
                                         addr_space="Shared",
                                         tag=f"fullx{_r}", name=f"fullx{_r}")
                          nc.gpsimd.collective_compute(
                              "AllGather", mybir.AluOpType.bypass,
                              replica_groups=rg,
                              ins=[shard[0][:]], outs=[fx[:]])

              # ---- message-passing layer ----
              self_qn = [0]
              def mp_layer(lname, fbuf, H, qrow, epilogue):
                  gcol = 0   # global chunk cursor (stream order)
                  with nc.named_scope(lname):
                      for g in range(NGRP):
                          blocks = list(range(g * GRPB, (g + 1) * GRPB))
                          KAg = sum(kA[b] for b in blocks)
                          KBg = sum(kB[b] for b in blocks)
                          Kg = KAg + KBg
                          gt = gpool.tile([P, KGmax, P], dt.bfloat16, tag="g")
                          # multi-packet gathers: one big call per group side
                          # keeps the Q7 fixed dispatch cost (~2.5us/call) off
                          # the critical path; descriptors stream through the
                          # SWDGE ring as SDMA drains it.
                          MAXC = 64
                          do_g = mpv in ("full", "g")
                          do_s = mpv in ("full", "sm", "s")
                          do_m = mpv in ("full", "sm")
                          for c0 in range(0, KAg if do_g else 0, MAXC):
                              nch = min(MAXC, KAg - c0)
                              o8 = (gcol + c0) * 8
                              nc.gpsimd.dma_gather(
                                  gt[:, c0:c0 + nch, :], fbuf[:],
                                  ix_t[:, o8:o8 + nch * 8],
                                  nch * P, nch * P, P,
                                  single_packet=False,
                                  queue_num=self_qn[0] % NQL)
                              self_qn[0] += 1
                          for c0 in range(0, KBg if do_g else 0, MAXC):
                              nch = min(MAXC, KBg - c0)
                              o8 = (gcol + KAg + c0) * 8
                              nc.gpsimd.dma_gather(
                                  gt[:, KAg + c0:KAg + c0 + nch, :],
                                  fbuf[BBASE:, :],
                                  ix_t[:, o8:o8 + nch * 8],
                                  nch * P, nch * P, P,
                                  single_packet=False,
                                  queue_num=self_qn[0] % NQL)
                              self_qn[0] += 1
                          # per-block segment sums
                          aoff = 0
                          boff = KAg
                          for b in blocks:
                              if not do_s:
                                  continue
                              ka, kb = kA[b], kB[b]
                              S = spool.tile([P, (ka + kb) * P], dt.bfloat16,
                                             tag="s")
                              s3 = S[:].rearrange("p (c q) -> p c q", q=P)
                              ca = gcol + (aoff)          # dstl col of A-run
                              cb = gcol + KAg + (boff - KAg)  # dstl col of B-run
                              if ka:
                                  nc.vector.tensor_tensor(
                                      s3[:, 0:ka, :],
                                      dl_t[:, ca:ca + ka].to_broadcast([P, ka, P]),
                                      io3.to_broadcast([P, ka, P]),
                                      op=mybir.AluOpType.is_equal)
                              if kb:
                                  nc.vector.tensor_tensor(
                                      s3[:, ka:ka + kb, :],
                                      dl_t[:, cb:cb + kb].to_broadcast([P, kb, P]),
                                      io3.to_broadcast([P, kb, P]),
                                      op=mybir.AluOpType.is_equal)
                              if not do_m:
                                  aoff += ka
                                  boff += kb
                                  continue
                              ps = pp.tile([P, H1], dt.float32, tag="mp")
                              psv = ps[:, 0:H]
                              first = True
                              if qrow is not None:
                                  nc.tensor.matmul(
                                      psv,
                                      iv_t[0:1, b * P:(b + 1) * P],
                                      qrow[0:1, :], start=True, stop=False)
                                  first = False
                              nch = ka + kb
                              for c in range(ka):
                                  nc.tensor.matmul(
                                      psv, s3[:, c, :], gt[:, aoff + c, 0:H],
                                      start=first and c == 0,
                                      stop=(c == nch - 1))
                              for c in range(kb):
                                  nc.tensor.matmul(
                                      psv, s3[:, ka + c, :], gt[:, boff + c, 0:H],
                                      start=first and ka == 0 and c == 0,
                                      stop=(ka + c == nch - 1))
                              epilogue(b, psv)
                              aoff += ka
                              boff += kb
                          gcol += Kg

              # ---- epilogues ----
              def epi1(b, psv):
                  tmp = work.tile([P, H1], dt.float32, tag="ep")
                  d = di_t[:, b:b + 1]
                  nc.scalar.activation(tmp[:], psv, Relu, scale=d)
                  nc.scalar.activation(A1[:, b * P:(b + 1) * P], tmp[:], Copy,
                                       scale=d)

              epi2_st = {}

              def epi2(b, psv):
                  g, j = b // GRPB, b % GRPB
                  if j == 0:
                      epi2_st["t"] = work.tile([P, GRPB, P], dt.bfloat16,
                                               tag="a2b", name="a2b")
                      nc.vector.memset(epi2_st["t"][:, :, H2:P], 0.0)
                  a2b = epi2_st["t"]
                  tmp = work.tile([P, H2], dt.float32, tag="ep")
                  d = di_t[:, b:b + 1]
                  nc.scalar.activation(tmp[:], psv, Relu, scale=d)
                  nc.scalar.activation(a2b[:, j, 0:H2], tmp[:], Copy, scale=d)
                  if j == GRPB - 1:
                      nc.sync.dma_start(
                          out=shard[0][g * GRPB * P:(g + 1) * GRPB * P, :]
                          .rearrange("(b p) h -> p b h", p=P),
                          in_=a2b[:])

              def epi3(b, psv):
                  r = work.tile([P, H2], dt.float32, tag="ep")
                  nc.scalar.activation(r[:], psv, Copy, scale=di_t[:, b:b + 1])
                  pst = pp.tile([H2, P], dt.float32, tag="tr")
                  nc.tensor.transpose(pst[:], r[:], idn_f[:])
                  rT = work.tile([H2, P], dt.float32, tag="rT")
                  nc.scalar.activation(rT[:], pst[:], Copy)
                  ps3 = pp.tile([P, NCLS], dt.float32, tag="o3")
                  nc.tensor.matmul(ps3[:], rT[:], w3_t[:], start=True, stop=False)
                  nc.tensor.matmul(ps3[:], on_t[0:1, :], b3_t[0:1, :],
                                   start=False, stop=True)
                  nc.scalar.activation(outacc[:, b * NCLS:(b + 1) * NCLS],
                                       ps3[:], Copy)

              if stages >= 3:
                  for _r in range(reps.get("mp1", 1)):
                      mp_layer("mp1", full[0], H1, q1_t, epi1)

              # ---- phase M2: h2 = a1 @ W2s -> shard1 ----
              if stages >= 4:
                with nc.named_scope("m2"):
                 for _r in range(reps.get("m2", 1)):
                  for g in range(NGRP):
                      h2b = work.tile([P, GRPB, P], dt.bfloat16, tag="h2b")
                      nc.vector.memset(h2b[:, :, H2:P], 0.0)
                      for j in range(GRPB):
                          b = g * GRPB + j
                          pst = pp.tile([P, P], dt.bfloat16, tag="tr")
                          nc.tensor.transpose(pst[:], A1[:, b * P:(b + 1) * P],
                                              idn_b[:])
                          a1T = work.tile([P, P], dt.bfloat16, tag="a1T")
                          nc.scalar.activation(a1T[:], pst[:], Copy)
                          ps2 = pp.tile([P, H1], dt.float32, tag="mp")
                          nc.tensor.matmul(ps2[:, 0:H2], a1T[:], w2_t[:],
                                           start=True, stop=True)
                          nc.scalar.activation(h2b[:, j, 0:H2], ps2[:, 0:H2],
                                               Copy)
                      nc.sync.dma_start(
                          out=shard[1][g * GRPB * P:(g + 1) * GRPB * P, :]
                          .rearrange("(b p) h -> p b h", p=P),
                          in_=h2b[:])
                  if stages >= 5:
                      allgather(1, 1)

              if stages >= 6:
                  mp_layer("mp2", full[1], H2, q2_t, epi2)
              if stages >= 7:
                  with nc.named_scope("ag3"):
                      allgather(0, 2)
              if stages >= 8:
                  mp_layer("mp3", full[2], H2, None, epi3)

            with nc.named_scope("fin"):
                nc.sync.dma_start(
                    out=outt.ap().rearrange("(b p) c -> p b c", p=P),
                    in_=outacc[:].rearrange("p (b c) -> p b c", c=NCLS))

    nc.compile()
    global _dbg_names
    _dbg_names = {"full": [f.tensor.name for f in full],
                  "shard": [f.tensor.name for f in shard]}
    return nc


def _run(inputs, trace=False):
    x = np.asarray(inputs["x"], np.float32)
    edge_index = np.asarray(inputs["edge_index"])
    key = hash(edge_index.tobytes())
    if key not in _cache:
        sched = _make_schedule(edge_index)
        sched["dst_all"] = np.concatenate(
            [edge_index[1], np.arange(N, dtype=np.int64)]).astype(np.int64)
        nc = _build(sched)
        _cache[key] = (sched, nc)
    sched, nc = _cache[key]
    sched["dst_all"] = np.concatenate(
        [edge_index[1], np.arange(N, dtype=np.int64)]).astype(np.int64)

    in_maps = _prep_inputs(
        sched, x,
        np.asarray(inputs["w1"], np.float32), np.asarray(inputs["b1"], np.float32),
        np.asarray(inputs["g1"], np.float32), np.asarray(inputs["beta1"], np.float32),
        np.asarray(inputs["m1"], np.float32), np.asarray(inputs["v1"], np.float32),
        np.asarray(inputs["w2"], np.float32), np.asarray(inputs["b2"], np.float32),
        np.asarray(inputs["g2"], np.float32), np.asarray(inputs["beta2"], np.float32),
        np.asarray(inputs["m2"], np.float32), np.asarray(inputs["v2"], np.float32),
        np.asarray(inputs["w3"], np.float32), np.asarray(inputs["b3"], np.float32),
    )
    kw = {}
    if trace:
        kw = dict(trace=True, trace_cores=list(range(NCORES)))
    res = run_bass_kernel_spmd(nc, in_maps, core_ids=list(range(NCORES)), **kw)
    out = np.concatenate([res.results[c]["out"] for c in range(NCORES)], axis=0)
    return out[:N].astype(np.float32), res


def kernel(**inputs) -> np.ndarray:
    out, _ = _run(inputs, trace=False)
    return out



# revision 6
# speedup vs baseline: 1.3765x; 1.3765x over previous
"""DeepGCN (3-layer GCNConv + BN + ReLU) on 8 Trainium2 NeuronCores.

Strategy (graph/data parallel, dst-sharded):
 - Nodes padded to NPAD=50176 = 8 cores x 6272 rows = 392 blocks of 128.
 - Edges (incl. self-loops) partitioned by destination core/block.
 - Per layer: sharded feature matmul -> AllGather of the (pre-scaled,
   bf16) message operand -> per-dst-block message passing:
     gather source rows (GPSIMD dma_gather, 256B rows) ->
     segment-sum via PE matmul with a 0/1 selection matrix built on DVE
     (is_equal of dst_local against an iota row) accumulated in PSUM ->
     ACT epilogue (Relu/scale by D^-1/2, BN affine folded into weights).
 - Normalization dis[src]*dis[dst] is separable: pre-scale the operand
   rows by dis, post-scale the segment sum by dis.
 - BN: y = s*conv + t with s,t folded into W (columns) and a rank-1 bias
   matmul (invdis x Q) so that dis*(segsum + invdis*Q) = dis*segsum + Q.
 - int16 gather indices (max 32767) => per-block edges split into an
   A call (src < 32768, base row 0) and a B call (src >= 17408, base row
   17408); the middle band is assigned to whichever side has slots.
"""

import numpy as np
import ml_dtypes

import concourse.bacc as bacc
import concourse.mybir as mybir
import concourse.tile as tile
from concourse.bass_utils import run_bass_kernel_spmd

BF16 = ml_dtypes.bfloat16

N = 50000
DIN = 512
H1 = 128
H2 = 64
NCLS = 2
EPS = 1e-5

NCORES = 8
P = 128
NB = 49                  # dst blocks per core
SHN = NB * P             # 6272 nodes per core
NPAD = NCORES * SHN      # 50176
GRPB = 7                 # blocks per gather group
NGRP = NB // GRPB        # 7
BBASE = 17408            # base row of the B gather window
AHI = 32768              # A window is [0, 32768)
NQ = 4                   # SWDGE queues for dma_gather

_cache: dict = {}
_dbg_names: dict = {}


# --------------------------------------------------------------------------
# host-side preprocessing
# --------------------------------------------------------------------------

def _make_schedule(edge_index):
    """Partition edges by (core, block), pick shared per-block chunk counts.

    Returns sched dict:
      kA, kB: [NB] shared chunk counts per block (A / B gather calls)
      per-core padded edge streams: gidx (int16 gather idx), dloc (f32 dst
      local, -1 for padding), laid out group-major:
      [g0: A-chunks(b0..b6) | B-chunks(b0..b6)] [g1: ...]
    """
    src = np.concatenate([edge_index[0], np.arange(N, dtype=np.int64)])
    dst = np.concatenate([edge_index[1], np.arange(N, dtype=np.int64)])
    src = src.astype(np.int64)
    dst = dst.astype(np.int64)

    core = dst // SHN
    blk = (dst % SHN) // P
    dl = (dst % P).astype(np.int32)
    key = (core * NB + blk).astype(np.int64)
    order = np.argsort(key, kind="stable")
    s_src = src[order]
    s_dl = dl[order]
    s_key = key[order]
    bounds = np.searchsorted(s_key, np.arange(NCORES * NB + 1))

    # segment (c,b): s_src[bounds[c*NB+b]:bounds[c*NB+b+1]]
    # per-core low/mid/high counts per block
    nlow = np.zeros((NCORES, NB), np.int64)
    nhigh = np.zeros((NCORES, NB), np.int64)
    ntot = np.zeros((NCORES, NB), np.int64)
    segs = {}
    for c in range(NCORES):
        for b in range(NB):
            i0, i1 = bounds[c * NB + b], bounds[c * NB + b + 1]
            ss = s_src[i0:i1]
            dd = s_dl[i0:i1]
            segs[(c, b)] = (ss, dd)
            ntot[c, b] = i1 - i0
            nlow[c, b] = int((ss < BBASE).sum())
            nhigh[c, b] = int((ss >= AHI).sum())

    K = np.maximum(1, -(-ntot.max(axis=0) // P))          # ceil
    kA_min = -(-nlow.max(axis=0) // P)
    kA_max = K - (-(-nhigh.max(axis=0) // P))
    assert (kA_min <= kA_max).all(), "A/B split infeasible"
    kA = np.clip((K * 2) // 3, kA_min, kA_max)
    kB = K - kA

    # build per-core streams
    gidx_cores, dloc_cores = [], []
    for c in range(NCORES):
        gparts, dparts = [], []
        for g in range(NGRP):
            blocks = range(g * GRPB, (g + 1) * GRPB)
            for side in (0, 1):
                for b in blocks:
                    ss, dd = segs[(c, b)]
                    low = ss < BBASE
                    high = ss >= AHI
                    mid = ~low & ~high
                    slots_a = int(kA[b]) * P
                    # A: all low + as many mid as fit
                    mid_idx = np.nonzero(mid)[0]
                    a_take = min(len(mid_idx), slots_a - int(low.sum()))
                    assert a_take >= 0
                    a_sel = np.concatenate([np.nonzero(low)[0], mid_idx[:a_take]])
                    b_sel = np.concatenate([mid_idx[a_take:], np.nonzero(high)[0]])
                    assert len(b_sel) <= int(kB[b]) * P
                    if side == 0:
                        sel, slots, base = a_sel, slots_a, 0
                    else:
                        sel, slots, base = b_sel, int(kB[b]) * P, BBASE
                    idx = ss[sel] - base
                    dloc = dd[sel].astype(np.float32)
                    padn = slots - len(sel)
                    idx = np.concatenate([idx, np.zeros(padn, np.int64)])
                    dloc = np.concatenate([dloc, -np.ones(padn, np.float32)])
                    assert (idx >= 0).all() and (idx < 32768).all()
                    gparts.append(idx.astype(np.int16))
                    dparts.append(dloc)
        gidx_cores.append(np.concatenate(gparts))
        dloc_cores.append(np.concatenate(dparts))

    T = int(K.sum())  # total chunks per core
    return {
        "kA": kA.astype(int).tolist(),
        "kB": kB.astype(int).tolist(),
        "T": T,
        "gidx": gidx_cores,
        "dloc": dloc_cores,
    }


def _prep_inputs(sched, x, w1, b1, g1, beta1, m1, v1,
                 w2, b2, g2, beta2, m2, v2, w3, b3):
    s1 = g1 / np.sqrt(v1 + EPS)
    t1 = beta1 - m1 * s1
    s2 = g2 / np.sqrt(v2 + EPS)
    t2 = beta2 - m2 * s2
    q1 = (s1 * b1 + t1).astype(np.float32)[None, :]
    q2 = (s2 * b2 + t2).astype(np.float32)[None, :]
    w1s = (w1 * s1[None, :]).astype(np.float32)
    w2s = (w2 * s2[None, :]).astype(np.float32)

    # degrees on A+I (in-degree by dst), dis = deg^-1/2
    deg = np.zeros(NPAD, np.float32)
    cnt = np.bincount(sched["dst_all"], minlength=N).astype(np.float32)
    deg[:N] = cnt
    dis = np.where(deg > 0, 1.0 / np.sqrt(np.maximum(deg, 1e-30)), 0.0)
    invdis = np.sqrt(deg)

    xp = np.zeros((NPAD, DIN), np.float32)
    xp[:N] = x

    T = sched["T"]
    iota = np.broadcast_to(np.arange(P, dtype=np.float32), (P, P))

    common = {
        "w1s": np.ascontiguousarray(
            w1s.reshape(4, P, H1).transpose(1, 0, 2)).astype(BF16),
        "w2s": w2s.astype(BF16),
        "w3": w3.astype(np.float32),
        "q1": q1,
        "q2": q2,
        "b3r": b3.astype(np.float32)[None, :],
        "ones": np.ones((1, P), np.float32),
        "iota": iota.astype(BF16),
    }

    in_maps = []
    for c in range(NCORES):
        off = c * SHN
        xc = xp[off:off + SHN].reshape(NB, P, 4, P)      # [b, n, t, p]
        xtt = np.ascontiguousarray(xc.transpose(0, 3, 2, 1)).astype(BF16)
        disc = np.ascontiguousarray(dis[off:off + SHN].reshape(NB, P).T)
        ivd = np.ascontiguousarray(invdis[off:off + SHN])[None, :]
        gidx = sched["gidx"][c]
        dloc = sched["dloc"][c]
        idx_sb = np.tile(gidx.reshape(T * 8, 16).T, (8, 1))
        dstl = np.ascontiguousarray(dloc.reshape(T, P).T).astype(BF16)
        m = dict(common)
        m.update({
            "xtt": xtt,
            "disc": disc,
            "ivd": ivd,
            "idx": np.ascontiguousarray(idx_sb),
            "dstl": dstl,
        })
        in_maps.append(m)
    return in_maps


# --------------------------------------------------------------------------
# bass program
# --------------------------------------------------------------------------

def _build(sched, stages=99, no_coll=False, reps=None, mpv="full", nq=None,
           small_coll=False):
    reps = reps or {}
    nq = NQ if nq is None else nq
    NQL = nq
    dt = mybir.dt
    kA, kB, T = sched["kA"], sched["kB"], sched["T"]
    K = [a + b for a, b in zip(kA, kB)]
    KGmax = max(sum(K[g * GRPB:(g + 1) * GRPB]) for g in range(NGRP))
    KBmax = max(K)

    nc = bacc.Bacc("TRN2", target_bir_lowering=False, debug=False,
                   num_devices=NCORES, num_swdge_queues=nq)

    xtt = nc.dram_tensor("xtt", [NB, P, 4, P], dt.bfloat16, kind="ExternalInput")
    w1s = nc.dram_tensor("w1s", [P, 4, H1], dt.bfloat16, kind="ExternalInput")
    w2s = nc.dram_tensor("w2s", [P, H2], dt.bfloat16, kind="ExternalInput")
    w3 = nc.dram_tensor("w3", [H2, NCLS], dt.float32, kind="ExternalInput")
    q1 = nc.dram_tensor("q1", [1, H1], dt.float32, kind="ExternalInput")
    q2 = nc.dram_tensor("q2", [1, H2], dt.float32, kind="ExternalInput")
    b3r = nc.dram_tensor("b3r", [1, NCLS], dt.float32, kind="ExternalInput")
    ones = nc.dram_tensor("ones", [1, P], dt.float32, kind="ExternalInput")
    iota = nc.dram_tensor("iota", [P, P], dt.bfloat16, kind="ExternalInput")
    disc = nc.dram_tensor("disc", [P, NB], dt.float32, kind="ExternalInput")
    ivd = nc.dram_tensor("ivd", [1, SHN], dt.float32, kind="ExternalInput")
    dstl = nc.dram_tensor("dstl", [P, T], dt.bfloat16, kind="ExternalInput")
    idx = nc.dram_tensor("idx", [P, T * 8], dt.int16, kind="ExternalInput")
    outt = nc.dram_tensor("out", [SHN, NCLS], dt.float32, kind="ExternalOutput")

    Relu = mybir.ActivationFunctionType.Relu
    Copy = mybir.ActivationFunctionType.Copy
    rg = [list(range(NCORES))]

    with tile.TileContext(nc) as tc:
        with (
            tc.tile_pool(name="cst", bufs=1) as cst,
            tc.tile_pool(name="res", bufs=1) as res,
            tc.tile_pool(name="dram", bufs=1, space="DRAM") as dram,
            tc.tile_pool(name="work", bufs=3) as work,
            tc.tile_pool(name="gt", bufs=2) as gpool,
            tc.tile_pool(name="sp", bufs=3) as spool,
            tc.tile_pool(name="ps", bufs=2, space="PSUM") as pp,
        ):
            # ---- constants into SBUF ----
            def cload(ap_dram, shape, dtype, tag):
                t = cst.tile(shape, dtype, tag=tag)
                nc.sync.dma_start(out=t[:], in_=ap_dram)
                return t

            w1_t = cload(w1s[:], [P, 4, H1], dt.bfloat16, "w1")
            w2_t = cload(w2s[:], [P, H2], dt.bfloat16, "w2")
            w3_t = cload(w3[:], [H2, NCLS], dt.float32, "w3")
            q1_t = cload(q1[:], [1, H1], dt.float32, "q1")
            q2_t = cload(q2[:], [1, H2], dt.float32, "q2")
            b3_t = cload(b3r[:], [1, NCLS], dt.float32, "b3")
            on_t = cload(ones[:], [1, P], dt.float32, "on")
            io_t = cload(iota[:], [P, P], dt.bfloat16, "io")
            di_t = cload(disc[:], [P, NB], dt.float32, "di")
            iv_t = cload(ivd[:], [1, SHN], dt.float32, "iv")
            dl_t = cload(dstl[:], [P, T], dt.bfloat16, "dl")
            ix_t = cload(idx[:], [P, T * 8], dt.int16, "ix")
            from concourse.masks import make_identity
            idn_f = cst.tile([P, P], dt.float32, tag="idf")
            make_identity(nc, idn_f[:])
            idn_b = cst.tile([P, P], dt.bfloat16, tag="idb")
            make_identity(nc, idn_b[:])

            io3 = io_t[:].rearrange("p (a q) -> p a q", a=1)

            # persistent tiles
            A1 = res.tile([P, NB * P], dt.bfloat16, tag="a1")
            outacc = res.tile([P, NB * NCLS], dt.float32, tag="oa")

            # dram bounce buffers (ping-pong)
            shard = [dram.tile([SHN, P], dt.bfloat16, tag=f"shard{i}", name=f"shard{i}")
                     for i in range(2)]

            def allgather(si, fi):
                if no_coll:
                    # timeline-sim stand-in: local copy only (timing analysis)
                    nc.sync.dma_start(out=full[fi][0:SHN, :], in_=shard[si][:])
                    return
                if small_coll:
                    # timing probe: 16-row collective (results garbage)
                    nc.gpsimd.collective_compute(
                        "AllGather", mybir.AluOpType.bypass, replica_groups=rg,
                        ins=[shard[si][0:16, :]], outs=[full[fi][0:128, :]],
                    )
                    return
                nc.gpsimd.collective_compute(
                    "AllGather", mybir.AluOpType.bypass, replica_groups=rg,
                    ins=[shard[si][:]], outs=[full[fi][:]],
                )

            for _rr in range(reps.get("all", 1)):
              full = [
                  dram.tile([NPAD, P], dt.bfloat16, addr_space="Shared",
                            tag=f"full{_rr}_{i}", name=f"full{_rr}_{i}")
                  for i in range(3)
              ]
              nc.vector.memset(outacc[:], 0.0)
              # ---- phase M1: h1 = dis * (x @ W1s) -> shard0 ----
              # batched per group of GRPB blocks to amortize SP DMA issue cost
              with nc.named_scope("m1"):
                for _r in range(reps.get("m1", 1)):
                  for g in range(NGRP):
                      xt = work.tile([P, GRPB, 4, P], dt.bfloat16, tag="xtt")
                      nc.sync.dma_start(
                          out=xt[:], in_=xtt[g * GRPB:(g + 1) * GRPB]
                          .rearrange("b p t n -> p b t n"))
                      hb = work.tile([P, GRPB, H1], dt.bfloat16, tag="hb")
                      for j in range(GRPB):
                          b = g * GRPB + j
                          ps = pp.tile([P, H1], dt.float32, tag="mp")
                          for t in range(4):
                              nc.tensor.matmul(ps[:], xt[:, j, t, :],
                                               w1_t[:, t, :],
                                               start=(t == 0), stop=(t == 3))
                          nc.scalar.activation(hb[:, j, :], ps[:], Copy,
                                               scale=di_t[:, b:b + 1])
                      nc.sync.dma_start(
                          out=shard[0][g * GRPB * P:(g + 1) * GRPB * P, :]
                          .rearrange("(b p) h -> p b h", p=P),
                          in_=hb[:])
                  if stages >= 2:
                      allgather(0, 0)
                      for _r in range(reps.get("ag1", 1) - 1):
                          fx = dram.tile([NPAD, P], dt.bfloat16,
                                         addr_space="Shared",
                                         tag=f"fullx{_r}", name=f"fullx{_r}")
                          nc.gpsimd.collective_compute(
                              "AllGather", mybir.AluOpType.bypass,
                              replica_groups=rg,
                              ins=[shard[0][:]], outs=[fx[:]])

              # ---- message-passing layer ----
              self_qn = [0]
              def mp_layer(lname, fbuf, H, qrow, epilogue):
                  gcol = 0   # global chunk cursor (stream order)
                  with nc.named_scope(lname):
                      for g in range(NGRP):
                          blocks = list(range(g * GRPB, (g + 1) * GRPB))
                          KAg = sum(kA[b] for b in blocks)
                          KBg = sum(kB[b] for b in blocks)
                          Kg = KAg + KBg
                          gt = gpool.tile([P, KGmax, P], dt.bfloat16, tag="g")
                          # dma_gather is limited to 1024 idxs (64 descs/packet
                          # x 16 engines) per call; split into <=8-chunk calls.
                          MAXC = 8
                          do_g = mpv in ("full", "g")
                          do_s = mpv in ("full", "sm", "s")
                          do_m = mpv in ("full", "sm")
                          for c0 in range(0, KAg if do_g else 0, MAXC):
                              nch = min(MAXC, KAg - c0)
                              o8 = (gcol + c0) * 8
                              nc.gpsimd.dma_gather(
                                  gt[:, c0:c0 + nch, :], fbuf[:],
                                  ix_t[:, o8:o8 + nch * 8],
                                  nch * P, nch * P, P,
                                  queue_num=self_qn[0] % NQL)
                              self_qn[0] += 1
                          for c0 in range(0, KBg if do_g else 0, MAXC):
                              nch = min(MAXC, KBg - c0)
                              o8 = (gcol + KAg + c0) * 8
                              nc.gpsimd.dma_gather(
                                  gt[:, KAg + c0:KAg + c0 + nch, :],
                                  fbuf[BBASE:, :],
                                  ix_t[:, o8:o8 + nch * 8],
                                  nch * P, nch * P, P,
                                  queue_num=self_qn[0] % NQL)
                              self_qn[0] += 1
                          # per-block segment sums
                          aoff = 0
                          boff = KAg
                          for b in blocks:
                              if not do_s:
                                  continue
                              ka, kb = kA[b], kB[b]
                              S = spool.tile([P, (ka + kb) * P], dt.bfloat16,
                                             tag="s")
                              s3 = S[:].rearrange("p (c q) -> p c q", q=P)
                              ca = gcol + (aoff)          # dstl col of A-run
                              cb = gcol + KAg + (boff - KAg)  # dstl col of B-run
                              if ka:
                                  nc.vector.tensor_tensor(
                                      s3[:, 0:ka, :],
                                      dl_t[:, ca:ca + ka].to_broadcast([P, ka, P]),
                                      io3.to_broadcast([P, ka, P]),
                                      op=mybir.AluOpType.is_equal)
                              if kb:
                                  nc.vector.tensor_tensor(
                                      s3[:, ka:ka + kb, :],
                                      dl_t[:, cb:cb + kb].to_broadcast([P, kb, P]),
                                      io3.to_broadcast([P, kb, P]),
                                      op=mybir.AluOpType.is_equal)
                              if not do_m:
                                  aoff += ka
                                  boff += kb
                                  continue
                              ps = pp.tile([P, H1], dt.float32, tag="mp")
                              psv = ps[:, 0:H]
                              first = True
                              if qrow is not None:
                                  nc.tensor.matmul(
                                      psv,
                                      iv_t[0:1, b * P:(b + 1) * P],
                                      qrow[0:1, :], start=True, stop=False)
                                  first = False
                              nch = ka + kb
                              for c in range(ka):
                                  nc.tensor.matmul(
                                      psv, s3[:, c, :], gt[:, aoff + c, 0:H],
                                      start=first and c == 0,
                                      stop=(c == nch - 1))
                              for c in range(kb):
                                  nc.tensor.matmul(
                                      psv, s3[:, ka + c, :], gt[:, boff + c, 0:H],
                                      start=first and ka == 0 and c == 0,
                                      stop=(ka + c == nch - 1))
                              epilogue(b, psv)
                              aoff += ka
                              boff += kb
                          gcol += Kg

              # ---- epilogues ----
              def epi1(b, psv):
                  tmp = work.tile([P, H1], dt.float32, tag="ep")
                  d = di_t[:, b:b + 1]
                  nc.scalar.activation(tmp[:], psv, Relu, scale=d)
                  nc.scalar.activation(A1[:, b * P:(b + 1) * P], tmp[:], Copy,
                                       scale=d)

              epi2_st = {}

              def epi2(b, psv):
                  g, j = b // GRPB, b % GRPB
                  if j == 0:
                      epi2_st["t"] = work.tile([P, GRPB, P], dt.bfloat16,
                                               tag="a2b", name="a2b")
                      nc.vector.memset(epi2_st["t"][:, :, H2:P], 0.0)
                  a2b = epi2_st["t"]
                  tmp = work.tile([P, H2], dt.float32, tag="ep")
                  d = di_t[:, b:b + 1]
                  nc.scalar.activation(tmp[:], psv, Relu, scale=d)
                  nc.scalar.activation(a2b[:, j, 0:H2], tmp[:], Copy, scale=d)
                  if j == GRPB - 1:
                      nc.sync.dma_start(
                          out=shard[0][g * GRPB * P:(g + 1) * GRPB * P, :]
                          .rearrange("(b p) h -> p b h", p=P),
                          in_=a2b[:])

              def epi3(b, psv):
                  r = work.tile([P, H2], dt.float32, tag="ep")
                  nc.scalar.activation(r[:], psv, Copy, scale=di_t[:, b:b + 1])
                  pst = pp.tile([H2, P], dt.float32, tag="tr")
                  nc.tensor.transpose(pst[:], r[:], idn_f[:])
                  rT = work.tile([H2, P], dt.float32, tag="rT")
                  nc.scalar.activation(rT[:], pst[:], Copy)
                  ps3 = pp.tile([P, NCLS], dt.float32, tag="o3")
                  nc.tensor.matmul(ps3[:], rT[:], w3_t[:], start=True, stop=False)
                  nc.tensor.matmul(ps3[:], on_t[0:1, :], b3_t[0:1, :],
                                   start=False, stop=True)
                  nc.scalar.activation(outacc[:, b * NCLS:(b + 1) * NCLS],
                                       ps3[:], Copy)

              if stages >= 3:
                  for _r in range(reps.get("mp1", 1)):
                      mp_layer("mp1", full[0], H1, q1_t, epi1)

              # ---- phase M2: h2 = a1 @ W2s -> shard1 ----
              if stages >= 4:
                with nc.named_scope("m2"):
                 for _r in range(reps.get("m2", 1)):
                  for g in range(NGRP):
                      h2b = work.tile([P, GRPB, P], dt.bfloat16, tag="h2b")
                      nc.vector.memset(h2b[:, :, H2:P], 0.0)
                      for j in range(GRPB):
                          b = g * GRPB + j
                          pst = pp.tile([P, P], dt.bfloat16, tag="tr")
                          nc.tensor.transpose(pst[:], A1[:, b * P:(b + 1) * P],
                                              idn_b[:])
                          a1T = work.tile([P, P], dt.bfloat16, tag="a1T")
                          nc.scalar.activation(a1T[:], pst[:], Copy)
                          ps2 = pp.tile([P, H1], dt.float32, tag="mp")
                          nc.tensor.matmul(ps2[:, 0:H2], a1T[:], w2_t[:],
                                           start=True, stop=True)
                          nc.scalar.activation(h2b[:, j, 0:H2], ps2[:, 0:H2],
                                               Copy)
                      nc.sync.dma_start(
                          out=shard[1][g * GRPB * P:(g + 1) * GRPB * P, :]
                          .rearrange("(b p) h -> p b h", p=P),
                          in_=h2b[:])
                  if stages >= 5:
                      allgather(1, 1)

              if stages >= 6:
                  mp_layer("mp2", full[1], H2, q2_t, epi2)
              if stages >= 7:
                  with nc.named_scope("ag3"):
                      allgather(0, 2)
              if stages >= 8:
                  mp_layer("mp3", full[2], H2, None, epi3)

            with nc.named_scope("fin"):
                nc.sync.dma_start(
                    out=outt.ap().rearrange("(b p) c -> p b c", p=P),
                    in_=outacc[:].rearrange("p (b c) -> p b c", c=NCLS))

    nc.compile()
    global _dbg_names
    _dbg_names = {"full": [f.tensor.name for f in full],
                  "shard": [f.tensor.name for f in shard]}
    return nc


def _run(inputs, trace=False):
    x = np.asarray(inputs["x"], np.float32)
    edge_index = np.asarray(inputs["edge_index"])
    key = hash(edge_index.tobytes())
    if key not in _cache:
        sched = _make_schedule(edge_index)
        sched["dst_all"] = np.concatenate(
            [edge_index[1], np.arange(N, dtype=np.int64)]).astype(np.int64)
        nc = _build(sched)
        _cache[key] = (sched, nc)
    sched, nc = _cache[key]
    sched["dst_all"] = np.concatenate(
        [edge_index[1], np.arange(N, dtype=np.int64)]).astype(np.int64)

    in_maps = _prep_inputs(
        sched, x,
        np.asarray(inputs["w1"], np.float32), np.asarray(inputs["b1"], np.float32),
        np.asarray(inputs["g1"], np.float32), np.asarray(inputs["beta1"], np.float32),
        np.asarray(inputs["m1"], np.float32), np.asarray(inputs["v1"], np.float32),
        np.asarray(inputs["w2"], np.float32), np.asarray(inputs["b2"], np.float32),
        np.asarray(inputs["g2"], np.float32), np.asarray(inputs["beta2"], np.float32),
        np.asarray(inputs["m2"], np.float32), np.asarray(inputs["v2"], np.float32),
        np.asarray(inputs["w3"], np.float32), np.asarray(inputs["b3"], np.float32),
    )
    kw = {}
    if trace:
        kw = dict(trace=True, trace_cores=list(range(NCORES)))
    res = run_bass_kernel_spmd(nc, in_maps, core_ids=list(range(NCORES)), **kw)
    out = np.concatenate([res.results[c]["out"] for c in range(NCORES)], axis=0)
    return out[:N].astype(np.float32), res


def kernel(**inputs) -> np.ndarray:
    out, _ = _run(inputs, trace=False)
    return out



# revision 13
# speedup vs baseline: 1.4144x; 1.0275x over previous
"""DeepGCN (3-layer GCNConv + BN + ReLU) on 8 Trainium2 NeuronCores.

Strategy (graph/data parallel, dst-sharded):
 - Nodes padded to NPAD=50176 = 8 cores x 6272 rows = 392 blocks of 128.
 - Edges (incl. self-loops) partitioned by destination core/block.
 - Per layer: sharded feature matmul -> AllGather of the (pre-scaled,
   bf16) message operand -> per-dst-block message passing:
     gather source rows (GPSIMD dma_gather, 256B rows) ->
     segment-sum via PE matmul with a 0/1 selection matrix built on DVE
     (is_equal of dst_local against an iota row) accumulated in PSUM ->
     ACT epilogue (Relu/scale by D^-1/2, BN affine folded into weights).
 - Normalization dis[src]*dis[dst] is separable: pre-scale the operand
   rows by dis, post-scale the segment sum by dis.
 - BN: y = s*conv + t with s,t folded into W (columns) and a rank-1 bias
   matmul (invdis x Q) so that dis*(segsum + invdis*Q) = dis*segsum + Q.
 - int16 gather indices (max 32767) => per-block edges split into an
   A call (src < 32768, base row 0) and a B call (src >= 17408, base row
   17408); the middle band is assigned to whichever side has slots.
 - Pipelining: AllGathers are sliced per block-group and issued as soon
   as the producing group's shard rows are written, so they overlap the
   producer phase; the tiny dense layer-2 matmul (m2) is interleaved
   into mp1's per-group loop to use idle PE/ACT slots there.
"""

import numpy as np
import ml_dtypes

import concourse.bacc as bacc
import concourse.mybir as mybir
import concourse.tile as tile
from concourse.bass_utils import run_bass_kernel_spmd

BF16 = ml_dtypes.bfloat16

N = 50000
DIN = 512
H1 = 128
H2 = 64
NCLS = 2
EPS = 1e-5

NCORES = 8
P = 128
NB = 49                  # dst blocks per core
SHN = NB * P             # 6272 nodes per core
NPAD = NCORES * SHN      # 50176
GRPB = 7                 # blocks per gather group
NGRP = NB // GRPB        # 7
BBASE = 17408            # base row of the B gather window
AHI = 32768              # A window is [0, 32768)
NQ = 4                   # SWDGE queues for dma_gather

_cache: dict = {}
_dbg_names: dict = {}


# --------------------------------------------------------------------------
# host-side preprocessing
# --------------------------------------------------------------------------

def _make_schedule(edge_index):
    """Partition edges by (core, block), pick shared per-block chunk counts.

    Returns sched dict:
      kA, kB: [NB] shared chunk counts per block (A / B gather calls)
      per-core padded edge streams: gidx (int16 gather idx), dloc (f32 dst
      local, -1 for padding), laid out group-major:
      [g0: A-chunks(b0..b6) | B-chunks(b0..b6)] [g1: ...]
    """
    src = np.concatenate([edge_index[0], np.arange(N, dtype=np.int64)])
    dst = np.concatenate([edge_index[1], np.arange(N, dtype=np.int64)])
    src = src.astype(np.int64)
    dst = dst.astype(np.int64)

    core = dst // SHN
    blk = (dst % SHN) // P
    dl = (dst % P).astype(np.int32)
    key = (core * NB + blk).astype(np.int64)
    order = np.argsort(key, kind="stable")
    s_src = src[order]
    s_dl = dl[order]
    s_key = key[order]
    bounds = np.searchsorted(s_key, np.arange(NCORES * NB + 1))

    # segment (c,b): s_src[bounds[c*NB+b]:bounds[c*NB+b+1]]
    # per-core low/mid/high counts per block
    nlow = np.zeros((NCORES, NB), np.int64)
    nhigh = np.zeros((NCORES, NB), np.int64)
    ntot = np.zeros((NCORES, NB), np.int64)
    segs = {}
    for c in range(NCORES):
        for b in range(NB):
            i0, i1 = bounds[c * NB + b], bounds[c * NB + b + 1]
            ss = s_src[i0:i1]
            dd = s_dl[i0:i1]
            segs[(c, b)] = (ss, dd)
            ntot[c, b] = i1 - i0
            nlow[c, b] = int((ss < BBASE).sum())
            nhigh[c, b] = int((ss >= AHI).sum())

    K = np.maximum(1, -(-ntot.max(axis=0) // P))          # ceil
    kA_min = -(-nlow.max(axis=0) // P)
    kA_max = K - (-(-nhigh.max(axis=0) // P))
    assert (kA_min <= kA_max).all(), "A/B split infeasible"
    kA = np.clip((K * 2) // 3, kA_min, kA_max)
    kB = K - kA

    # build per-core streams
    gidx_cores, dloc_cores = [], []
    for c in range(NCORES):
        gparts, dparts = [], []
        for g in range(NGRP):
            blocks = range(g * GRPB, (g + 1) * GRPB)
            for side in (0, 1):
                for b in blocks:
                    ss, dd = segs[(c, b)]
                    low = ss < BBASE
                    high = ss >= AHI
                    mid = ~low & ~high
                    slots_a = int(kA[b]) * P
                    # A: all low + as many mid as fit
                    mid_idx = np.nonzero(mid)[0]
                    a_take = min(len(mid_idx), slots_a - int(low.sum()))
                    assert a_take >= 0
                    a_sel = np.concatenate([np.nonzero(low)[0], mid_idx[:a_take]])
                    b_sel = np.concatenate([mid_idx[a_take:], np.nonzero(high)[0]])
                    assert len(b_sel) <= int(kB[b]) * P
                    if side == 0:
                        sel, slots, base = a_sel, slots_a, 0
                    else:
                        sel, slots, base = b_sel, int(kB[b]) * P, BBASE
                    idx = ss[sel] - base
                    dloc = dd[sel].astype(np.float32)
                    padn = slots - len(sel)
                    idx = np.concatenate([idx, np.zeros(padn, np.int64)])
                    dloc = np.concatenate([dloc, -np.ones(padn, np.float32)])
                    assert (idx >= 0).all() and (idx < 32768).all()
                    gparts.append(idx.astype(np.int16))
                    dparts.append(dloc)
        gidx_cores.append(np.concatenate(gparts))
        dloc_cores.append(np.concatenate(dparts))

    T = int(K.sum())  # total chunks per core
    return {
        "kA": kA.astype(int).tolist(),
        "kB": kB.astype(int).tolist(),
        "T": T,
        "gidx": gidx_cores,
        "dloc": dloc_cores,
    }


def _prep_inputs(sched, x, w1, b1, g1, beta1, m1, v1,
                 w2, b2, g2, beta2, m2, v2, w3, b3):
    s1 = g1 / np.sqrt(v1 + EPS)
    t1 = beta1 - m1 * s1
    s2 = g2 / np.sqrt(v2 + EPS)
    t2 = beta2 - m2 * s2
    q1 = (s1 * b1 + t1).astype(np.float32)[None, :]
    q2 = (s2 * b2 + t2).astype(np.float32)[None, :]
    w1s = (w1 * s1[None, :]).astype(np.float32)
    w2s = (w2 * s2[None, :]).astype(np.float32)

    # degrees on A+I (in-degree by dst), dis = deg^-1/2
    deg = np.zeros(NPAD, np.float32)
    cnt = np.bincount(sched["dst_all"], minlength=N).astype(np.float32)
    deg[:N] = cnt
    dis = np.where(deg > 0, 1.0 / np.sqrt(np.maximum(deg, 1e-30)), 0.0)
    invdis = np.sqrt(deg)

    xp = np.zeros((NPAD, DIN), np.float32)
    xp[:N] = x

    T = sched["T"]
    iota = np.broadcast_to(np.arange(P, dtype=np.float32), (P, P))

    common = {
        "w1s": np.ascontiguousarray(
            w1s.reshape(4, P, H1).transpose(1, 0, 2)).astype(BF16),
        "w2s": w2s.astype(BF16),
        "w3": w3.astype(np.float32),
        "q1": q1,
        "q2": q2,
        "b3r": b3.astype(np.float32)[None, :],
        "ones": np.ones((1, P), np.float32),
        "iota": iota.astype(BF16),
    }

    in_maps = []
    for c in range(NCORES):
        off = c * SHN
        xc = xp[off:off + SHN].reshape(NB, P, 4, P)      # [b, n, t, p]
        xtt = np.ascontiguousarray(xc.transpose(0, 3, 2, 1)).astype(BF16)
        disc = np.ascontiguousarray(dis[off:off + SHN].reshape(NB, P).T)
        ivd = np.ascontiguousarray(invdis[off:off + SHN])[None, :]
        gidx = sched["gidx"][c]
        dloc = sched["dloc"][c]
        idx_sb = np.tile(gidx.reshape(T * 8, 16).T, (8, 1))
        dstl = np.ascontiguousarray(dloc.reshape(T, P).T).astype(BF16)
        m = dict(common)
        m.update({
            "xtt": xtt,
            "disc": disc,
            "ivd": ivd,
            "idx": np.ascontiguousarray(idx_sb),
            "dstl": dstl,
        })
        in_maps.append(m)
    return in_maps


# --------------------------------------------------------------------------
# bass program
# --------------------------------------------------------------------------

def _build(sched, gt_bufs=3):
    dt = mybir.dt
    kA, kB, T = sched["kA"], sched["kB"], sched["T"]
    K = [a + b for a, b in zip(kA, kB)]
    KGmax = max(sum(K[g * GRPB:(g + 1) * GRPB]) for g in range(NGRP))

    nc = bacc.Bacc("TRN2", target_bir_lowering=False, debug=False,
                   num_devices=NCORES, num_swdge_queues=NQ)

    xtt = nc.dram_tensor("xtt", [NB, P, 4, P], dt.bfloat16, kind="ExternalInput")
    w1s = nc.dram_tensor("w1s", [P, 4, H1], dt.bfloat16, kind="ExternalInput")
    w2s = nc.dram_tensor("w2s", [P, H2], dt.bfloat16, kind="ExternalInput")
    w3 = nc.dram_tensor("w3", [H2, NCLS], dt.float32, kind="ExternalInput")
    q1 = nc.dram_tensor("q1", [1, H1], dt.float32, kind="ExternalInput")
    q2 = nc.dram_tensor("q2", [1, H2], dt.float32, kind="ExternalInput")
    b3r = nc.dram_tensor("b3r", [1, NCLS], dt.float32, kind="ExternalInput")
    ones = nc.dram_tensor("ones", [1, P], dt.float32, kind="ExternalInput")
    iota = nc.dram_tensor("iota", [P, P], dt.bfloat16, kind="ExternalInput")
    disc = nc.dram_tensor("disc", [P, NB], dt.float32, kind="ExternalInput")
    ivd = nc.dram_tensor("ivd", [1, SHN], dt.float32, kind="ExternalInput")
    dstl = nc.dram_tensor("dstl", [P, T], dt.bfloat16, kind="ExternalInput")
    idx = nc.dram_tensor("idx", [P, T * 8], dt.int16, kind="ExternalInput")
    outt = nc.dram_tensor("out", [SHN, NCLS], dt.float32, kind="ExternalOutput")

    Relu = mybir.ActivationFunctionType.Relu
    Copy = mybir.ActivationFunctionType.Copy
    rg = [list(range(NCORES))]

    with tile.TileContext(nc) as tc:
        with (
            tc.tile_pool(name="cst", bufs=1) as cst,
            tc.tile_pool(name="res", bufs=1) as res,
            tc.tile_pool(name="dram", bufs=1, space="DRAM") as dram,
            tc.tile_pool(name="work", bufs=3) as work,
            tc.tile_pool(name="gt", bufs=gt_bufs) as gpool,
            tc.tile_pool(name="sp", bufs=3) as spool,
            tc.tile_pool(name="ps", bufs=2, space="PSUM") as pp,
        ):
            # ---- constants into SBUF ----
            def cload(ap_dram, shape, dtype, tag):
                t = cst.tile(shape, dtype, tag=tag)
                nc.sync.dma_start(out=t[:], in_=ap_dram)
                return t

            w1_t = cload(w1s[:], [P, 4, H1], dt.bfloat16, "w1")
            w2_t = cload(w2s[:], [P, H2], dt.bfloat16, "w2")
            w3_t = cload(w3[:], [H2, NCLS], dt.float32, "w3")
            q1_t = cload(q1[:], [1, H1], dt.float32, "q1")
            q2_t = cload(q2[:], [1, H2], dt.float32, "q2")
            b3_t = cload(b3r[:], [1, NCLS], dt.float32, "b3")
            on_t = cload(ones[:], [1, P], dt.float32, "on")
            io_t = cload(iota[:], [P, P], dt.bfloat16, "io")
            di_t = cload(disc[:], [P, NB], dt.float32, "di")
            iv_t = cload(ivd[:], [1, SHN], dt.float32, "iv")
            dl_t = cload(dstl[:], [P, T], dt.bfloat16, "dl")
            ix_t = cload(idx[:], [P, T * 8], dt.int16, "ix")
            from concourse.masks import make_identity
            idn_f = cst.tile([P, P], dt.float32, tag="idf")
            make_identity(nc, idn_f[:])
            idn_b = cst.tile([P, P], dt.bfloat16, tag="idb")
            make_identity(nc, idn_b[:])

            io3 = io_t[:].rearrange("p (a q) -> p a q", a=1)

            # persistent tiles
            A1 = res.tile([P, NB * P], dt.bfloat16, tag="a1")
            outacc = res.tile([P, NB * NCLS], dt.float32, tag="oa")

            # dram bounce buffers (ping-pong)
            shard = [dram.tile([SHN, P], dt.bfloat16, tag=f"shard{i}",
                               name=f"shard{i}")
                     for i in range(2)]
            full = [
                dram.tile([NPAD, P], dt.bfloat16, addr_space="Shared",
                          tag=f"full{i}", name=f"full{i}")
                for i in range(3)
            ]

            def allgather(si, fi):
                nc.gpsimd.collective_compute(
                    "AllGather", mybir.AluOpType.bypass, replica_groups=rg,
                    ins=[shard[si][:]], outs=[full[fi][:]],
                )

            nc.vector.memset(outacc[:], 0.0)

            # ---- phase M1: h1 = dis * (x @ W1s) -> shard0 ----
            # batched per group of GRPB blocks to amortize SP DMA issue cost
            with nc.named_scope("m1"):
                for g in range(NGRP):
                    xt = work.tile([P, GRPB, 4, P], dt.bfloat16, tag="xtt")
                    nc.sync.dma_start(
                        out=xt[:], in_=xtt[g * GRPB:(g + 1) * GRPB]
                        .rearrange("b p t n -> p b t n"))
                    hb = work.tile([P, GRPB, H1], dt.bfloat16, tag="hb")
                    for j in range(GRPB):
                        b = g * GRPB + j
                        ps = pp.tile([P, H1], dt.float32, tag="mp")
                        for t in range(4):
                            nc.tensor.matmul(ps[:], xt[:, j, t, :],
                                             w1_t[:, t, :],
                                             start=(t == 0), stop=(t == 3))
                        nc.scalar.activation(hb[:, j, :], ps[:], Copy,
                                             scale=di_t[:, b:b + 1])
                    nc.sync.dma_start(
                        out=shard[0][g * GRPB * P:(g + 1) * GRPB * P, :]
                        .rearrange("(b p) h -> p b h", p=P),
                        in_=hb[:])
                allgather(0, 0)

            # ---- message-passing layer ----
            self_qn = [0]

            def mp_layer(lname, fbuf, H, qrow, epilogue, post_group=None):
                gcol = 0   # global chunk cursor (stream order)
                with nc.named_scope(lname):
                    for g in range(NGRP):
                        blocks = list(range(g * GRPB, (g + 1) * GRPB))
                        KAg = sum(kA[b] for b in blocks)
                        KBg = sum(kB[b] for b in blocks)
                        Kg = KAg + KBg
                        gt = gpool.tile([P, KGmax, P], dt.bfloat16, tag="g")
                        # dma_gather is limited to 1024 idxs (64 descs/packet
                        # x 16 engines) per call; split into <=8-chunk calls.
                        MAXC = 8
                        for c0 in range(0, KAg, MAXC):
                            nch = min(MAXC, KAg - c0)
                            o8 = (gcol + c0) * 8
                            nc.gpsimd.dma_gather(
                                gt[:, c0:c0 + nch, :], fbuf[:],
                                ix_t[:, o8:o8 + nch * 8],
                                nch * P, nch * P, P,
                                queue_num=self_qn[0] % NQ)
                            self_qn[0] += 1
                        for c0 in range(0, KBg, MAXC):
                            nch = min(MAXC, KBg - c0)
                            o8 = (gcol + KAg + c0) * 8
                            nc.gpsimd.dma_gather(
                                gt[:, KAg + c0:KAg + c0 + nch, :],
                                fbuf[BBASE:, :],
                                ix_t[:, o8:o8 + nch * 8],
                                nch * P, nch * P, P,
                                queue_num=self_qn[0] % NQ)
                            self_qn[0] += 1
                        # per-block segment sums
                        aoff = 0
                        boff = KAg
                        for b in blocks:
                            ka, kb = kA[b], kB[b]
                            S = spool.tile([P, (ka + kb) * P], dt.bfloat16,
                                           tag="s")
                            s3 = S[:].rearrange("p (c q) -> p c q", q=P)
                            ca = gcol + aoff              # dstl col of A-run
                            cb = gcol + boff              # dstl col of B-run
                            if ka:
                                nc.vector.tensor_tensor(
                                    s3[:, 0:ka, :],
                                    dl_t[:, ca:ca + ka].to_broadcast([P, ka, P]),
                                    io3.to_broadcast([P, ka, P]),
                                    op=mybir.AluOpType.is_equal)
                            if kb:
                                nc.vector.tensor_tensor(
                                    s3[:, ka:ka + kb, :],
                                    dl_t[:, cb:cb + kb].to_broadcast([P, kb, P]),
                                    io3.to_broadcast([P, kb, P]),
                                    op=mybir.AluOpType.is_equal)
                            ps = pp.tile([P, H1], dt.float32, tag="mp")
                            psv = ps[:, 0:H]
                            first = True
                            if qrow is not None:
                                nc.tensor.matmul(
                                    psv,
                                    iv_t[0:1, b * P:(b + 1) * P],
                                    qrow[0:1, :], start=True, stop=False)
                                first = False
                            nch = ka + kb
                            for c in range(ka):
                                nc.tensor.matmul(
                                    psv, s3[:, c, :], gt[:, aoff + c, 0:H],
                                    start=first and c == 0,
                                    stop=(c == nch - 1))
                            for c in range(kb):
                                nc.tensor.matmul(
                                    psv, s3[:, ka + c, :], gt[:, boff + c, 0:H],
                                    start=first and ka == 0 and c == 0,
                                    stop=(ka + c == nch - 1))
                            epilogue(b, psv)
                            aoff += ka
                            boff += kb
                        gcol += Kg
                        if post_group is not None:
                            post_group(g)

            # ---- epilogues ----
            def epi1(b, psv):
                tmp = work.tile([P, H1], dt.float32, tag="ep")
                d = di_t[:, b:b + 1]
                nc.scalar.activation(tmp[:], psv, Relu, scale=d)
                nc.scalar.activation(A1[:, b * P:(b + 1) * P], tmp[:], Copy,
                                     scale=d)

            # m2 for one group, interleaved into mp1 (uses idle PE/ACT there)
            def m2_group(g):
                with nc.named_scope("m2"):
                    h2b = work.tile([P, GRPB, P], dt.bfloat16, tag="h2b")
                    nc.vector.memset(h2b[:, :, H2:P], 0.0)
                    for j in range(GRPB):
                        b = g * GRPB + j
                        pst = pp.tile([P, P], dt.bfloat16, tag="tr")
                        nc.tensor.transpose(pst[:], A1[:, b * P:(b + 1) * P],
                                            idn_b[:])
                        a1T = work.tile([P, P], dt.bfloat16, tag="a1T")
                        nc.scalar.activation(a1T[:], pst[:], Copy)
                        ps2 = pp.tile([P, H1], dt.float32, tag="mp")
                        nc.tensor.matmul(ps2[:, 0:H2], a1T[:], w2_t[:],
                                         start=True, stop=True)
                        nc.scalar.activation(h2b[:, j, 0:H2], ps2[:, 0:H2],
                                             Copy)
                    nc.sync.dma_start(
                        out=shard[1][g * GRPB * P:(g + 1) * GRPB * P, :]
                        .rearrange("(b p) h -> p b h", p=P),
                        in_=h2b[:])
                    if g == NGRP - 1:
                        allgather(1, 1)

            epi2_st = {}

            def epi2(b, psv):
                g, j = b // GRPB, b % GRPB
                if j == 0:
                    epi2_st["t"] = work.tile([P, GRPB, P], dt.bfloat16,
                                             tag="a2b", name="a2b")
                    nc.vector.memset(epi2_st["t"][:, :, H2:P], 0.0)
                a2b = epi2_st["t"]
                tmp = work.tile([P, H2], dt.float32, tag="ep")
                d = di_t[:, b:b + 1]
                nc.scalar.activation(tmp[:], psv, Relu, scale=d)
                nc.scalar.activation(a2b[:, j, 0:H2], tmp[:], Copy, scale=d)
                if j == GRPB - 1:
                    nc.sync.dma_start(
                        out=shard[0][g * GRPB * P:(g + 1) * GRPB * P, :]
                        .rearrange("(b p) h -> p b h", p=P),
                        in_=a2b[:])

            def epi3(b, psv):
                r = work.tile([P, H2], dt.float32, tag="ep")
                nc.scalar.activation(r[:], psv, Copy, scale=di_t[:, b:b + 1])
                pst = pp.tile([H2, P], dt.float32, tag="tr")
                nc.tensor.transpose(pst[:], r[:], idn_f[:])
                rT = work.tile([H2, P], dt.float32, tag="rT")
                nc.scalar.activation(rT[:], pst[:], Copy)
                ps3 = pp.tile([P, NCLS], dt.float32, tag="o3")
                nc.tensor.matmul(ps3[:], rT[:], w3_t[:], start=True, stop=False)
                nc.tensor.matmul(ps3[:], on_t[0:1, :], b3_t[0:1, :],
                                 start=False, stop=True)
                nc.scalar.activation(outacc[:, b * NCLS:(b + 1) * NCLS],
                                     ps3[:], Copy)

            mp_layer("mp1", full[0], H1, q1_t, epi1, post_group=m2_group)
            mp_layer("mp2", full[1], H2, q2_t, epi2,
                     post_group=lambda g: (allgather(0, 2)
                                           if g == NGRP - 1 else None))
            mp_layer("mp3", full[2], H2, None, epi3)

            with nc.named_scope("fin"):
                nc.sync.dma_start(
                    out=outt.ap().rearrange("(b p) c -> p b c", p=P),
                    in_=outacc[:].rearrange("p (b c) -> p b c", c=NCLS))

    nc.compile()
    global _dbg_names
    _dbg_names = {"full": [f.tensor.name for f in full],
                  "shard": [f.tensor.name for f in shard]}
    return nc


def _run(inputs, trace=False):
    x = np.asarray(inputs["x"], np.float32)
    edge_index = np.asarray(inputs["edge_index"])
    key = hash(edge_index.tobytes())
    if key not in _cache:
        sched = _make_schedule(edge_index)
        sched["dst_all"] = np.concatenate(
            [edge_index[1], np.arange(N, dtype=np.int64)]).astype(np.int64)
        try:
            nc = _build(sched, gt_bufs=3)
        except Exception:
            nc = _build(sched, gt_bufs=2)
        _cache[key] = (sched, nc)
    sched, nc = _cache[key]
    sched["dst_all"] = np.concatenate(
        [edge_index[1], np.arange(N, dtype=np.int64)]).astype(np.int64)

    in_maps = _prep_inputs(
        sched, x,
        np.asarray(inputs["w1"], np.float32), np.asarray(inputs["b1"], np.float32),
        np.asarray(inputs["g1"], np.float32), np.asarray(inputs["beta1"], np.float32),
        np.asarray(inputs["m1"], np.float32), np.asarray(inputs["v1"], np.float32),
        np.asarray(inputs["w2"], np.float32), np.asarray(inputs["b2"], np.float32),
        np.asarray(inputs["g2"], np.float32), np.asarray(inputs["beta2"], np.float32),
        np.asarray(inputs["m2"], np.float32), np.asarray(inputs["v2"], np.float32),
        np.asarray(inputs["w3"], np.float32), np.asarray(inputs["b3"], np.float32),
    )
    kw = {}
    if trace:
        kw = dict(trace=True, trace_cores=list(range(NCORES)))
    res = run_bass_kernel_spmd(nc, in_maps, core_ids=list(range(NCORES)), **kw)
    out = np.concatenate([res.results[c]["out"] for c in range(NCORES)], axis=0)
    return out[:N].astype(np.float32), res


def kernel(**inputs) -> np.ndarray:
    out, _ = _run(inputs, trace=False)
    return out
